# revision 8
# baseline (speedup 1.0000x reference)
"""Correlation-layer cosine-similarity kernel for Trainium2 (8 NeuronCores), v2.

Problem: x1, x2: [B=4, C=256, H=128, W=256] fp32.
out[b, d, h, w] = cos-sim over C of (x1[b,:,h,w], x2_padded[b,:,h,w+d]), d in 0..40.

Sharding: core i handles batch b = i//2 and H-half hh = i%2 (64 rows).

Structure: work is batched in h-row blocks (sizes 4,8,...,8,4 -- tapered so
the pipeline fills and drains faster) and software-pipelined: the next block's
inputs prefetch while this block computes, and each block's post-staging tail
(normalize/transpose/output) is deferred into the next iteration. Per block:
  - inputs arrive via casting f32->bf16 DMAs on the Pool (SWDGE) queue;
  - squares on the vector engine (bf16 at 2x);
  - channel norms as batched PSUM *columns* (tiny stationary matmuls against
    a ones vector; n2 uses three overlapping 128-wide windows so every column
    is full height), then one sqrt (scalar) + one fast reciprocal (vector);
  - the 1/n2 columns are PE-transposed to rows and staged to DRAM (rn2d);
  - per row, the Gram band cover G[w, w'] ([128 x 336], the two 168-wide
    mc-halves) accumulates in PSUM over two matmuls each; the scalar engine
    drains it to bf16 SBUF scaled by 1/n1 (per-partition scalar);
  - the cover is staged to DRAM (gd) and the 41 diagonals per row come back
    in one skewed-AP DMA per mc; a second skewed *broadcast* read of rn2d
    returns the d-shifted 1/n2 aligned elementwise with the band;
  - tail (next iteration): one elementwise multiply applies 1/n2, the PE
    transposes each row into [41, 2, 128] (partition = d), scalar/vector
    engines drain PSUM, and a single 3-dim DMA writes the block's output
    (bf16; the host converts to f32).
DMA roles are split across engine queues (Pool: inputs, SP: staging + output,
Act: small rn2d tail) and emitted in per-queue ready-time order to avoid
head-of-line blocking on the in-order sequencers. Note: GPSIMD must never
touch PSUM (walrus rejects it even though CoreSim allows it).
"""

import numpy as np

B, C, H, W = 4, 256, 128, 256
D = 41           # displacements 0..40
HC = 64          # H rows per core
PAD = 40
W2 = W + PAD     # 296
COV = 336        # staged cover cols per h: 2 mc-blocks x 168
HB = 8           # max h rows per block
NB = HC // HB    # 8 blocks

_cache = {}
TRACE = False    # test-harness knob; harness never sets it


def _build_nc():
    import concourse.bass as bass
    import concourse.tile as tile
    from concourse import bacc, mybir
    from concourse.masks import make_identity

    f32 = mybir.dt.float32
    bf16 = mybir.dt.bfloat16
    mult = mybir.AluOpType.mult

    nc = bacc.Bacc(trn_type="TRN2")
    x1s = nc.dram_tensor("x1s", [C, HC, W], f32, kind="ExternalInput")
    x2s = nc.dram_tensor("x2s", [C, HC, W], f32, kind="ExternalInput")
    outs = nc.dram_tensor("outs", [D, HC, W], bf16, kind="ExternalOutput")
    # DRAM scratch (full-size, no reuse hazards):
    # gd: per h a [128 w, 336] bf16 band-cover rectangle (already scaled by
    #     1/n1); diagonals read back with a skewed AP.
    # rn2d: per h the 1/n2 row over w' in [0, 296) (bf16), read back with a
    #     skewed broadcast AP.
    gd = nc.dram_tensor("gd", [HC, 128, COV], bf16, kind="Internal")
    rn2d = nc.dram_tensor("rn2d", [HC, COV], bf16, kind="Internal")

    with tile.TileContext(nc) as tc:
        with (
            tc.tile_pool(name="const", bufs=1) as constp,
            tc.tile_pool(name="io", bufs=3) as io,
            tc.tile_pool(name="sqp", bufs=2) as sqp,
            tc.tile_pool(name="gsbp", bufs=2) as gsbp,
            tc.tile_pool(name="bandp", bufs=2) as bandp,
            tc.tile_pool(name="outp", bufs=2) as outp,
            tc.tile_pool(name="rnp", bufs=2) as rnp,
            tc.tile_pool(name="gp", bufs=3, space="PSUM") as gp,
            tc.tile_pool(name="npp", bufs=1, space="PSUM") as npp,
            tc.tile_pool(name="tpp", bufs=3, space="PSUM") as tpp,
            tc.tile_pool(name="trp", bufs=1, space="PSUM") as trp,
        ):
            ones_col = constp.tile([128, 1], bf16)
            nc.vector.memset(ones_col, 1.0)
            epsb = constp.tile([128, 1], f32)
            nc.vector.memset(epsb, 1e-12)
            ident = constp.tile([128, 128], bf16)
            make_identity(nc, ident)

            def load_block(h0, hb):
                x1blk = io.tile([128, 2, HB, W], bf16, tag="x1blk")
                x2blk = io.tile([128, 2, HB, W2], bf16, tag="x2blk")
                nc.vector.memset(x2blk[:, :, 0:hb, W:W2], 0.0)
                nc.gpsimd.dma_start(
                    out=x1blk[:, :, 0:hb, :],
                    in_=bass.AP(tensor=x1s, offset=h0 * W,
                                ap=[[HC * W, 128], [128 * HC * W, 2],
                                    [1, hb * W]]),
                )
                for kc in range(2):
                    nc.gpsimd.dma_start(
                        out=x2blk[:, kc, 0:hb, 0:W],
                        in_=bass.AP(tensor=x2s,
                                    offset=kc * 128 * HC * W + h0 * W,
                                    ap=[[HC * W, 128], [W, hb], [1, W]]),
                    )
                return x1blk, x2blk

            def tail_block(h0, hb, band, n2sk):
                """Deferred back half of a block (emitted after the next
                block's staging DMAs so the SP queue stays monotone in
                ready-time): apply the n2 factor, PE-transpose each row,
                collect and write the block output."""
                band_n = bandp.tile([128, HB, 2, D], bf16, tag="band_n")
                nc.vector.tensor_tensor(out=band_n[:, 0:hb],
                                        in0=band[:, 0:hb],
                                        in1=n2sk[:, 0:hb], op=mult)
                # transpose per (j, mc) into [41, 2, 128] so the block output
                # needs only one 3-dim DMA (partition dim = d alone)
                out_sb = outp.tile([D, HB, 2, 128], bf16, tag="out_sb")
                for j0 in range(0, hb, 2):
                    jn = min(2, hb - j0)
                    tp = tpp.tile([D, 2, 2, 128], bf16, tag="tp")
                    for jj in range(jn):
                        for mc in range(2):
                            nc.tensor.transpose(
                                tp[:, jj, mc, :],
                                band_n[:, j0 + jj, mc, :], ident)
                    if (j0 // 2) % 2 == 0:
                        nc.scalar.copy(out_sb[:, j0:j0 + jn, :, :],
                                       tp[:, 0:jn])
                    else:
                        nc.vector.tensor_copy(out_sb[:, j0:j0 + jn, :, :],
                                              tp[:, 0:jn])
                nc.sync.dma_start(
                    out=bass.AP(tensor=outs, offset=h0 * W,
                                ap=[[HC * W, D], [W, hb], [1, 2 * 128]]),
                    in_=out_sb[:, 0:hb])

            sizes = [4] + [8] * 7 + [4]
            starts = [sum(sizes[:i]) for i in range(len(sizes))]
            pending = load_block(starts[0], sizes[0])
            tail = None
            for b in range(len(sizes)):
                h0, hb = starts[b], sizes[b]
                x1blk, x2blk = pending
                if b + 1 < len(sizes):
                    pending = load_block(starts[b + 1], sizes[b + 1])

                # squares (bf16, both on vector: 2x rate on 16-bit)
                sq1 = sqp.tile([128, 2, HB, W], bf16, tag="sq1")
                nc.vector.tensor_tensor(out=sq1[:, :, 0:hb], in0=x1blk[:, :, 0:hb],
                                        in1=x1blk[:, :, 0:hb], op=mult)
                sq2 = sqp.tile([128, 2, HB, W2], bf16, tag="sq2")
                nc.vector.tensor_tensor(out=sq2[:, :, 0:hb], in0=x2blk[:, :, 0:hb],
                                        in1=x2blk[:, :, 0:hb], op=mult)

                # batched norm matmuls, all as [128, 1] columns: per h --
                #   q=0,1: n1sq for w = mc*128 + p
                #   q=2,3,4: n2sq for w' windows [0:128], [128:256], [168:296]
                ncolp = npp.tile([128, 5, HB], f32, tag="ncolp")
                n2w = [(0, 128), (128, 256), (168, W2)]
                for j in range(hb):
                    for mc in range(2):
                        for kc in range(2):
                            nc.tensor.matmul(
                                ncolp[:, mc, j:j + 1],
                                sq1[:, kc, j, mc * 128:(mc + 1) * 128],
                                ones_col,
                                start=(kc == 0), stop=(kc == 1))
                    for gidx, (w0, w1) in enumerate(n2w):
                        for kc in range(2):
                            nc.tensor.matmul(
                                ncolp[:, 2 + gidx, j:j + 1],
                                sq2[:, kc, j, w0:w1],
                                ones_col,
                                start=(kc == 0), stop=(kc == 1))

                # sqrt (scalar) then fast reciprocal (vector), batched
                ncol = rnp.tile([128, 5, HB], f32, tag="ncol")
                nc.scalar.activation(
                    out=ncol[:, :, 0:hb], in_=ncolp[:, :, 0:hb],
                    func=mybir.ActivationFunctionType.Sqrt,
                    bias=epsb, scale=1.0)
                rncol = rnp.tile([128, 5, HB], f32, tag="rncol")
                nc.vector.reciprocal_approx_fast(out=rncol[:, :, 0:hb],
                                                 in_=ncol[:, :, 0:hb])

                # transpose the 1/n2 columns into rows [3*hb, 128] and stage
                # them to DRAM as per-h rows rn2d[h, 0:296]
                rn2bf = rnp.tile([128, 3 * HB], bf16, tag="rn2bf")
                for gidx in range(3):
                    nc.vector.tensor_copy(
                        rn2bf[:, gidx * hb:(gidx + 1) * hb],
                        rncol[:, 2 + gidx, 0:hb])
                tp_rn = trp.tile([3 * HB, 128], bf16, tag="tp_rn")
                nc.tensor.transpose(tp_rn[0:3 * hb, :], rn2bf[:, 0:3 * hb],
                                    ident)
                rn2row = rnp.tile([3 * HB, 128], bf16, tag="rn2row")
                nc.vector.tensor_copy(rn2row[0:3 * hb, :], tp_rn[0:3 * hb, :])
                # rows (g, j): g0 -> w' 0:128, g1 -> 128:256, g2 cols 88:128
                # -> w' 256:296
                nc.sync.dma_start(
                    out=bass.AP(tensor=rn2d, offset=h0 * COV,
                                ap=[[128, 2], [COV, hb], [1, 128]]),
                    in_=rn2row[0:2 * hb, :])
                nc.scalar.dma_start(
                    out=bass.AP(tensor=rn2d, offset=h0 * COV + 256,
                                ap=[[COV, hb], [1, PAD]]),
                    in_=rn2row[2 * hb:3 * hb, 88:128])

                # per h: Gram band cover into PSUM, scale by 1/n1 on the way
                # to the bf16 staging tile (scalar engine)
                gsb = gsbp.tile([128, HB, COV], bf16, tag="gsb")
                for j in range(hb):
                    g = gp.tile([128, COV], f32, tag="g")
                    for kc in range(2):
                        nc.tensor.matmul(g[:, 0:168],
                                         x1blk[:, kc, j, 0:128],
                                         x2blk[:, kc, j, 0:168],
                                         start=(kc == 0), stop=(kc == 1))
                    for kc in range(2):
                        nc.tensor.matmul(g[:, 168:COV],
                                         x1blk[:, kc, j, 128:256],
                                         x2blk[:, kc, j, 128:W2],
                                         start=(kc == 0), stop=(kc == 1))
                    for mc in range(2):
                        nc.scalar.activation(
                            out=gsb[:, j, mc * 168:(mc + 1) * 168],
                            in_=g[:, mc * 168:(mc + 1) * 168],
                            func=mybir.ActivationFunctionType.Copy,
                            scale=rncol[:, mc, j:j + 1])

                # stage cover to DRAM; read back the 41 diagonals per h and
                # the d-shifted 1/n2 values, both with skewed APs
                h1 = hb // 2
                nc.sync.dma_start(
                    out=bass.AP(tensor=gd, offset=h0 * 128 * COV,
                                ap=[[COV, 128], [128 * COV, h1], [1, COV]]),
                    in_=gsb[:, 0:h1])
                nc.sync.dma_start(
                    out=bass.AP(tensor=gd, offset=(h0 + h1) * 128 * COV,
                                ap=[[COV, 128], [128 * COV, hb - h1],
                                    [1, COV]]),
                    in_=gsb[:, h1:hb])
                band = bandp.tile([128, HB, 2, D], bf16, tag="band")
                n2sk = bandp.tile([128, HB, 2, D], bf16, tag="n2sk")
                for mc in range(2):
                    nc.sync.dma_start(
                        out=band[:, 0:hb, mc, :],
                        in_=bass.AP(tensor=gd,
                                    offset=h0 * 128 * COV + mc * 168,
                                    ap=[[COV + 1, 128], [128 * COV, hb],
                                        [1, D]]))
                    nc.sync.dma_start(
                        out=n2sk[:, 0:hb, mc, :],
                        in_=bass.AP(tensor=rn2d,
                                    offset=h0 * COV + mc * 128,
                                    ap=[[1, 128], [COV, hb], [1, D]]))

                # previous block's deferred back half
                if tail is not None:
                    tail_block(*tail)
                tail = (h0, hb, band, n2sk)
            tail_block(*tail)

    nc.finalize()
    return nc


def kernel(x_1: np.ndarray, x_2: np.ndarray) -> np.ndarray:
    from concourse.bass_utils import run_bass_kernel_spmd

    if "nc" not in _cache:
        _cache["nc"] = _build_nc()
    nc = _cache["nc"]

    x_1 = np.asarray(x_1, dtype=np.float32)
    x_2 = np.asarray(x_2, dtype=np.float32)

    in_maps = []
    for i in range(8):
        b, hh = i // 2, i % 2
        sl = slice(hh * HC, (hh + 1) * HC)
        in_maps.append({
            "x1s": np.ascontiguousarray(x_1[b, :, sl, :]),
            "x2s": np.ascontiguousarray(x_2[b, :, sl, :]),
        })
    res = run_bass_kernel_spmd(nc, in_maps, core_ids=list(range(8)), trace=TRACE)
    _cache["last_result"] = res
    out = np.empty((B, D, H, W), dtype=np.float32)
    for i in range(8):
        b, hh = i // 2, i % 2
        out[b, :, hh * HC:(hh + 1) * HC, :] = np.asarray(
            res.results[i]["outs"], dtype=np.float32)
    return out
